# revision 14
# baseline (speedup 1.0000x reference)
"""GroupSortActivation (GROUP_SIZE=2) Trainium2 Bass kernel.

out[:, 2i]   = min(x[:, 2i], x[:, 2i+1])
out[:, 2i+1] = max(x[:, 2i], x[:, 2i+1])

Sharding: batch dim (16384) split evenly across 8 NeuronCores (2048 rows
per core); no communication. Per core: stream tiles of (128, 4096) fp32,
two strided tensor_tensor ops (min/max) on DVE, stream back out.

Raw-bass pipeline (walrus limits attached sync-waits per instruction, so
all waits are standalone sequencer instructions):
  SP  (sync):   loads  x -> t[i%NB], slot gated on DVE progress
  DVE (vector): waits store-slot free + load done, then min/max
  ACT (scalar): stores o[i%NO] -> y, gated on DVE progress
Per-slot DMA-completion semaphores make out-of-order DMA completion safe.
"""

import numpy as np

import concourse.bass as bass
from concourse import mybir
from concourse.bass_utils import run_bass_kernel_spmd

N_CORES = 8
B, D = 16384, 4096
RPC = B // N_CORES  # rows per core = 2048
P = 128  # SBUF partitions
HSPLIT = 2  # column halves per row-block (1MB tiles: 8KB/partition contiguous)
FREE = D // HSPLIT  # free dim per tile = 2048
N_TILES = (RPC // P) * HSPLIT  # 32
NB = 8  # input slots  (8 x 1MB)
NO = 8  # output slots (8 x 1MB)


def build_nc() -> bass.Bass:
    nc = bass.Bass()
    x = nc.dram_tensor("x", [RPC, D], mybir.dt.float32, kind="ExternalInput")
    y = nc.dram_tensor("y", [RPC, D], mybir.dt.float32, kind="ExternalOutput")

    # tile q covers rows [(q//2)*128, ...+128), cols [(q%2)*2048, ...+2048)
    def xt(q):
        i, h = q // HSPLIT, q % HSPLIT
        return x[i * P : (i + 1) * P, h * FREE : (h + 1) * FREE]

    def yt(q):
        i, h = q // HSPLIT, q % HSPLIT
        return y[i * P : (i + 1) * P, h * FREE : (h + 1) * FREE]

    from contextlib import ExitStack

    with ExitStack() as ctx:
        t = [
            ctx.enter_context(nc.sbuf_tensor(f"t{j}", [P, FREE], mybir.dt.float32))
            for j in range(NB)
        ]
        o = [
            ctx.enter_context(nc.sbuf_tensor(f"o{k}", [P, FREE], mybir.dt.float32))
            for k in range(NO)
        ]
        ld = [ctx.enter_context(nc.semaphore(f"ld{j}")) for j in range(NB)]
        st = [ctx.enter_context(nc.semaphore(f"st{k}")) for k in range(NO)]
        dv = ctx.enter_context(nc.semaphore("dv"))

        block = ctx.enter_context(nc.Block())

        @block.sync
        def _(sync):
            for i in range(N_TILES):
                j = i % NB
                if i >= NB:
                    # input slot j free once tile i-NB's max (2 ops/tile) ran
                    sync.wait_ge(dv, 2 * (i - NB) + 2)
                sync.dma_start(t[j][:], xt(i)).then_inc(ld[j], 16)

        @block.vector
        def _(vector):
            for i in range(N_TILES):
                j, k = i % NB, i % NO
                if i >= NO:
                    # output slot k free once tile i-NO's store completed
                    vector.wait_ge(st[k], 16 * (i // NO))
                vector.wait_ge(ld[j], 16 * (i // NB + 1))
                te, to = t[j][:, 0::2], t[j][:, 1::2]
                vector.tensor_tensor(
                    o[k][:, 0::2], te, to, op=mybir.AluOpType.min
                ).then_inc(dv, 1)
                vector.tensor_tensor(
                    o[k][:, 1::2], te, to, op=mybir.AluOpType.max
                ).then_inc(dv, 1)

        @block.scalar
        def _(scalar):
            for i in range(N_TILES):
                k = i % NO
                scalar.wait_ge(dv, 2 * i + 2)
                scalar.dma_start(yt(i), o[k][:]).then_inc(st[k], 16)
            # make sure every store landed before the program ends
            for k in range(NO):
                uses = len([i for i in range(N_TILES) if i % NO == k])
                scalar.wait_ge(st[k], 16 * uses)

    return nc


_NC_CACHE = None


def _get_nc() -> bass.Bass:
    global _NC_CACHE
    if _NC_CACHE is None:
        _NC_CACHE = build_nc()
    return _NC_CACHE


def make_in_maps(x: np.ndarray) -> list[dict[str, np.ndarray]]:
    xs = np.ascontiguousarray(np.asarray(x), dtype=np.float32)
    assert xs.shape == (B, D), xs.shape
    return [{"x": xs[i * RPC : (i + 1) * RPC]} for i in range(N_CORES)]


def kernel(x: np.ndarray) -> np.ndarray:
    res = run_bass_kernel_spmd(_get_nc(), make_in_maps(x), list(range(N_CORES)))
    return np.concatenate([r["y"] for r in res.results], axis=0)


# revision 15
# speedup vs baseline: 1.1418x; 1.1418x over previous
"""GroupSortActivation (GROUP_SIZE=2) Trainium2 Bass kernel.

out[:, 2i]   = min(x[:, 2i], x[:, 2i+1])
out[:, 2i+1] = max(x[:, 2i], x[:, 2i+1])

Sharding: batch dim (16384) split evenly across 8 NeuronCores (2048 rows
per core); no communication. Per core: stream 16 tiles of (128, 4096)
fp32 (2MB, one DRAM row per partition = 16KB contiguous per partition),
two strided tensor_tensor ops (min/max) on DVE, stream back out.
Measured ~175us/core on HW = ~366 GB/s/core of the ~436 GB/s fabric cap;
DMA-bound with all 16 SDMA engines ~96% busy.

Raw-bass pipeline (walrus limits attached sync-waits per instruction —
TensorTensor allows only 1 and HWDGE DIRECT2D DMA allows none/one — so
all waits are standalone sequencer instructions):
  SP  (sync):   loads  x -> t[i%NB]  (HWDGE), slot gated on DVE progress
  DVE (vector): waits store-slot free + load done, then min/max
  ACT (scalar): stores o[i%NO] -> y  (HWDGE), gated on DVE progress
Per-slot DMA-completion semaphores make out-of-order DMA completion safe.
fp32 tensor_tensor runs in 1x DVE mode regardless of stride, so the
stride-2 access patterns cost nothing extra; compute (~70us/core) hides
entirely under DMA (~158us/core busy).
"""

import numpy as np

import concourse.bass as bass
from concourse import mybir
from concourse.bass_utils import run_bass_kernel_spmd

N_CORES = 8
B, D = 16384, 4096
RPC = B // N_CORES  # rows per core = 2048
P = 128  # SBUF partitions
N_TILES = RPC // P  # 16 tiles of (128, 4096)
NB = 4  # input slots  (4 x 2MB)
NO = 4  # output slots (4 x 2MB)


def build_nc() -> bass.Bass:
    nc = bass.Bass()
    x = nc.dram_tensor("x", [RPC, D], mybir.dt.float32, kind="ExternalInput")
    y = nc.dram_tensor("y", [RPC, D], mybir.dt.float32, kind="ExternalOutput")

    from contextlib import ExitStack

    with ExitStack() as ctx:
        t = [
            ctx.enter_context(nc.sbuf_tensor(f"t{j}", [P, D], mybir.dt.float32))
            for j in range(NB)
        ]
        o = [
            ctx.enter_context(nc.sbuf_tensor(f"o{k}", [P, D], mybir.dt.float32))
            for k in range(NO)
        ]
        ld = [ctx.enter_context(nc.semaphore(f"ld{j}")) for j in range(NB)]
        st = [ctx.enter_context(nc.semaphore(f"st{k}")) for k in range(NO)]
        dv = ctx.enter_context(nc.semaphore("dv"))

        block = ctx.enter_context(nc.Block())

        @block.sync
        def _(sync):
            for i in range(N_TILES):
                j = i % NB
                if i >= NB:
                    # input slot j free once tile i-NB's max (2 ops/tile) ran
                    sync.wait_ge(dv, 2 * (i - NB) + 2)
                sync.dma_start(t[j][:], x[i * P : (i + 1) * P, :]).then_inc(ld[j], 16)

        @block.vector
        def _(vector):
            for i in range(N_TILES):
                j, k = i % NB, i % NO
                if i >= NO:
                    # output slot k free once tile i-NO's store completed
                    vector.wait_ge(st[k], 16 * (i // NO))
                vector.wait_ge(ld[j], 16 * (i // NB + 1))
                te, to = t[j][:, 0::2], t[j][:, 1::2]
                vector.tensor_tensor(
                    o[k][:, 0::2], te, to, op=mybir.AluOpType.min
                ).then_inc(dv, 1)
                vector.tensor_tensor(
                    o[k][:, 1::2], te, to, op=mybir.AluOpType.max
                ).then_inc(dv, 1)

        @block.scalar
        def _(scalar):
            for i in range(N_TILES):
                k = i % NO
                scalar.wait_ge(dv, 2 * i + 2)
                scalar.dma_start(y[i * P : (i + 1) * P, :], o[k][:]).then_inc(
                    st[k], 16
                )
            # make sure every store landed before the program ends
            for k in range(NO):
                uses = len([i for i in range(N_TILES) if i % NO == k])
                scalar.wait_ge(st[k], 16 * uses)

    return nc


_NC_CACHE = None


def _get_nc() -> bass.Bass:
    global _NC_CACHE
    if _NC_CACHE is None:
        _NC_CACHE = build_nc()
    return _NC_CACHE


def make_in_maps(x: np.ndarray) -> list[dict[str, np.ndarray]]:
    xs = np.ascontiguousarray(np.asarray(x), dtype=np.float32)
    assert xs.shape == (B, D), xs.shape
    return [{"x": xs[i * RPC : (i + 1) * RPC]} for i in range(N_CORES)]


def kernel(x: np.ndarray) -> np.ndarray:
    res = run_bass_kernel_spmd(_get_nc(), make_in_maps(x), list(range(N_CORES)))
    return np.concatenate([r["y"] for r in res.results], axis=0)
